# revision 67
# baseline (speedup 1.0000x reference)
"""DeepSeekMoE Trainium2 kernel (8 NeuronCores, expert-parallel).

Design:
  - Core e owns routed expert e (dense over all T tokens; the combine weight
    c[t,e] zeros out tokens not routed to e, which matches the reference math
    exactly) plus a 128-wide I-slice of the shared expert.
  - All activations are kept transposed (feature-major, [feat, tokens]) so
    every matmul streams tokens as the moving operand with expert weights
    stationary.
  - Router (logits -> sigmoid -> top-2 -> combine weights) runs on-device in
    fp32 (exact); the heavy matmuls run in float32r (~10-bit mantissa,
    4x faster than fp32 on the PE).
  - Host only shards/reassembles: x transpose, per-core weight slices, final
    sum over the 8 partial outputs (each core's partial covers disjoint
    experts / I-slices, so the host sum is exact fp32 adds).
"""
import numpy as np
from contextlib import ExitStack

import concourse.bass as bass
import concourse.bacc as bacc
import concourse.tile as tile
from concourse import masks, mybir
from concourse.bass_utils import run_bass_kernel_spmd

F32 = mybir.dt.float32
F32R = mybir.dt.float32r
I32 = mybir.dt.int32

B, S, H, I, E = 2, 1024, 1024, 1024, 8
T = B * S
HC = H // 128          # contraction chunks for H
IC = I // 128          # I chunks (routed expert M-tiles)
MT = IC + 1            # M-tiles: 8 routed I-chunks + 1 shared-expert slice
TJ = T // 128          # 16 token tiles (for the router transpose)
NCORES = 8
TOPK = 2


def _emit(nc, tc, ctx):
    AluOp = mybir.AluOpType
    Act = mybir.ActivationFunctionType

    xT_d = nc.dram_tensor("xT", [H, T], F32, kind="ExternalInput").ap()
    rwg_d = nc.dram_tensor("rwg", [H, I], F32, kind="ExternalInput").ap()
    rwu_d = nc.dram_tensor("rwu", [H, I], F32, kind="ExternalInput").ap()
    rwd_d = nc.dram_tensor("rwd", [I, H], F32, kind="ExternalInput").ap()
    swg_d = nc.dram_tensor("swg", [H, 128], F32, kind="ExternalInput").ap()
    swu_d = nc.dram_tensor("swu", [H, 128], F32, kind="ExternalInput").ap()
    swd_d = nc.dram_tensor("swd", [128, H], F32, kind="ExternalInput").ap()
    gwT_d = nc.dram_tensor("gwT", [H, E], F32, kind="ExternalInput").ap()
    bt_d = nc.dram_tensor("biastie", [E], F32, kind="ExternalInput").ap()
    ar_d = nc.dram_tensor("arangeE", [E], F32, kind="ExternalInput").ap()
    oh_d = nc.dram_tensor("onehot", [E], F32, kind="ExternalInput").ap()

    outT_d = nc.dram_tensor("outT", [H, T], F32, kind="ExternalOutput").ap()
    idx_d = nc.dram_tensor("idx", [T, TOPK], I32, kind="ExternalOutput").ap()

    # ---------------- persistent pools ----------------
    big = ctx.enter_context(tc.tile_pool(name="big", bufs=1))
    wpool = ctx.enter_context(tc.tile_pool(name="w", bufs=4))
    wshp = ctx.enter_context(tc.tile_pool(name="wsh", bufs=3))
    spool = ctx.enter_context(tc.tile_pool(name="silu", bufs=3))
    dram = ctx.enter_context(tc.tile_pool(name="dram", bufs=1, space="DRAM"))
    # single shared PSUM pool: 8 x 1-bank slots, shared by router and main
    # stages so main matmuls can use banks the router isn't holding
    pp = ctx.enter_context(tc.tile_pool(name="ps", bufs=8, space="PSUM"))

    x32r = big.tile([128, HC, T], F32R, tag="x32r")
    a_sb = big.tile([128, MT, T], F32R, tag="A")
    cbc = big.tile([128, T], F32, tag="cbc")
    ident = big.tile([128, 128], F32, tag="ident")
    masks.make_identity(nc, ident[:])

    # ---------------- router (fp32, exact) ----------------
    # x is DMA'd once as fp32 chunks; the router consumes them directly and
    # the DVE converts each chunk into the resident f32r copy for the main
    # matmul stages. The shared-expert G accumulation rides along inside the
    # chunk loop so the PE has main-pipeline work while x streams.
    wg8 = wpool.tile([128, HC, 128], F32R, tag="w", name="wg8")
    wu8 = wpool.tile([128, HC, 128], F32R, tag="w", name="wu8")
    swg_r0 = swg_d.rearrange("(c p) i -> p c i", p=128)
    swu_r0 = swu_d.rearrange("(c p) i -> p c i", p=128)
    with tc.tile_wait_until(0.012):
        nc.sync.dma_start(wg8[:], swg_r0.bitcast(F32R))
        nc.sync.dma_start(wu8[:], swu_r0.bitcast(F32R))
    g8s = [pp.tile([128, 512], F32, tag="ps", name=f"g8_{n}") for n in range(4)]
    with ExitStack() as rctx:
        rsb = rctx.enter_context(tc.tile_pool(name="rsb", bufs=1))
        xcp = rctx.enter_context(tc.tile_pool(name="xc", bufs=4))

        # Router logits in f32r at fp32 accuracy via hi/lo splitting:
        #   logits = gwh.xh + gwh.xl + gwl.xh   (+gwl.xl ~ 1e-8, dropped)
        # where xh/gwh are the f32r-rounded values and xl/gwl the residuals.
        # Each f32r pass runs at 4x the fp32 matmul rate.
        gw_sb = rsb.tile([128, HC, E], F32, tag="gw")
        gwh = rsb.tile([128, HC, E], F32R, tag="gwh")
        gwl = rsb.tile([128, HC, E], F32R, tag="gwl")
        with tc.high_priority():
            nc.sync.dma_start(gw_sb[:], gwT_d.rearrange("(c p) e -> p c e", p=128))
            nc.sync.dma_start(gwh[:],
                              gwT_d.rearrange("(c p) e -> p c e", p=128).bitcast(F32R))
        nc.vector.tensor_tensor(gwl[:], gw_sb[:], gwh[:], AluOp.subtract)
        bt_sb = rsb.tile([128, E], F32, tag="bt")
        nc.sync.dma_start(bt_sb[:], bass.AP(bt_d.tensor, 0, [[0, 128], [1, E]]))
        ar_sb = rsb.tile([128, E], F32, tag="ar")
        nc.sync.dma_start(ar_sb[:], bass.AP(ar_d.tensor, 0, [[0, 128], [1, E]]))
        oh_sb = rsb.tile([128, E], F32, tag="oh")
        nc.sync.dma_start(oh_sb[:], bass.AP(oh_d.tensor, 0, [[0, 128], [1, E]]))

        # logits (E-major) accumulated over H chunks; x streamed in fp32
        scoE = rsb.tile([8, T], F32, tag="scoE")
        lps = [pp.tile([8, 512], F32, tag="ps", name=f"lp{n}") for n in range(4)]
        for k in range(HC):
            for hf in range(2):
                xc = xcp.tile([128, T // 2], F32, tag="xc")
                csl = slice(1024 * hf, 1024 * (hf + 1))
                with tc.high_priority() if k == 0 else ExitStack():
                    nc.sync.dma_start(xc[:], xT_d[128 * k:128 * (k + 1), csl])
                # f32r conversion for the main stages (rounds on write) on
                # the otherwise-idle GpSimd engine; the DVE only computes the
                # router residual xl = x - xh
                nc.gpsimd.tensor_copy(x32r[:, k, csl], xc[:])
                xlt = xcp.tile([128, T // 2], F32R, tag="xl", bufs=2)
                nc.vector.tensor_tensor(xlt[:], xc[:], x32r[:, k, csl],
                                        AluOp.subtract)
                for nn in range(2):
                    n = 2 * hf + nn
                    xh = x32r[:, k, 512 * (2 * hf + nn):512 * (2 * hf + nn + 1)]
                    xl = xlt[:, 512 * nn:512 * (nn + 1)]
                    nc.tensor.matmul(lps[n][:], gwh[:, k, :], xh,
                                     start=(k == 0), stop=False)
                    nc.tensor.matmul(lps[n][:], gwh[:, k, :], xl,
                                     start=False, stop=False)
                    nc.tensor.matmul(lps[n][:], gwl[:, k, :], xh,
                                     start=False, stop=(k == HC - 1))
            # shared-expert G accumulation rides the stream: PE work that is
            # ready as soon as this chunk's f32r conversion lands
            for n in range(4):
                nc.tensor.matmul(g8s[n][:], wg8[:, k, :],
                                 x32r[:, k, 512 * n:512 * (n + 1)],
                                 start=(k == 0), stop=(k == HC - 1))
        for n in range(4):
            nc.scalar.activation(scoE[:, 512 * n:512 * (n + 1)], lps[n][:],
                                 Act.Sigmoid)

        # transpose scores to token-major [128, TJ, E]
        sco = rsb.tile([128, TJ, E], F32, tag="sco")
        for j in range(TJ):
            tp = pp.tile([128, 8], F32, tag="ps", name=f"tp{j}")
            nc.tensor.transpose(tp[:], scoE[:, 128 * j:128 * (j + 1)],
                                ident[0:8, 0:8])
            nc.vector.tensor_copy(sco[:, j, :], tp[:])

        def vtile(tag):
            return rsb.tile([128, TJ], F32, tag=tag, name=tag)

        def etile(tag):
            return rsb.tile([128, TJ, E], F32, tag=tag, name=tag)

        def bcE(t):  # [128, TJ] -> [128, TJ, E] free-broadcast
            ap = t[:]
            return bass.AP(ap.tensor, ap.offset, ap.ap + [[0, E]])

        def bcTJ(t):  # [128, E] -> [128, TJ, E] free-broadcast
            ap = t[:]
            return bass.AP(ap.tensor, ap.offset,
                           [ap.ap[0], [0, TJ], ap.ap[1]])

        # Defer the whole top-2 DVE chain: c is only consumed by stage C's
        # combine (~150us in), and scheduling it early starves the DVE work
        # (silu products) that recycles the main pipeline's PSUM slots.
        tc.tile_set_cur_wait(0.085)
        sel = etile("sel")
        nc.vector.tensor_tensor(sel[:], sco[:], bcTJ(bt_sb), AluOp.add)

        max1 = vtile("max1")
        nc.vector.tensor_reduce(max1[:], sel[:], mybir.AxisListType.X, AluOp.max)
        eq1 = etile("eq1")
        nc.vector.tensor_tensor(eq1[:], sel[:], bcE(max1), AluOp.is_ge)
        tmp = etile("tmp")
        nc.vector.tensor_tensor(tmp[:], sco[:], eq1[:], AluOp.mult)
        s1 = vtile("s1")
        nc.vector.tensor_reduce(s1[:], tmp[:], mybir.AxisListType.X, AluOp.max)
        nc.vector.tensor_tensor(tmp[:], bcTJ(ar_sb), eq1[:], AluOp.mult)
        idx1 = vtile("idx1")
        nc.vector.tensor_reduce(idx1[:], tmp[:], mybir.AxisListType.X, AluOp.max)

        sel2 = etile("sel2")
        nc.vector.scalar_tensor_tensor(sel2[:], eq1[:], -1e30, sel[:],
                                       AluOp.mult, AluOp.add)
        max2 = vtile("max2")
        nc.vector.tensor_reduce(max2[:], sel2[:], mybir.AxisListType.X, AluOp.max)
        eq2 = etile("eq2")
        nc.vector.tensor_tensor(eq2[:], sel2[:], bcE(max2), AluOp.is_ge)
        nc.vector.tensor_tensor(tmp[:], sco[:], eq2[:], AluOp.mult)
        s2 = vtile("s2")
        nc.vector.tensor_reduce(s2[:], tmp[:], mybir.AxisListType.X, AluOp.max)
        nc.vector.tensor_tensor(tmp[:], bcTJ(ar_sb), eq2[:], AluOp.mult)
        idx2 = vtile("idx2")
        nc.vector.tensor_reduce(idx2[:], tmp[:], mybir.AxisListType.X, AluOp.max)

        den = vtile("den")
        nc.vector.tensor_tensor(den[:], s1[:], s2[:], AluOp.add)
        nc.vector.tensor_scalar_max(den[:], den[:], 1e-9)
        dinv = vtile("dinv")
        nc.vector.reciprocal(dinv[:], den[:])
        w1 = vtile("w1")
        nc.vector.tensor_tensor(w1[:], s1[:], dinv[:], AluOp.mult)
        w2 = vtile("w2")
        nc.vector.tensor_tensor(w2[:], s2[:], dinv[:], AluOp.mult)

        e1 = vtile("e1")
        nc.vector.tensor_tensor(tmp[:], eq1[:], bcTJ(oh_sb), AluOp.mult)
        nc.vector.tensor_reduce(e1[:], tmp[:], mybir.AxisListType.X, AluOp.max)
        e2 = vtile("e2")
        nc.vector.tensor_tensor(tmp[:], eq2[:], bcTJ(oh_sb), AluOp.mult)
        nc.vector.tensor_reduce(e2[:], tmp[:], mybir.AxisListType.X, AluOp.max)
        cE = vtile("cE")
        nc.vector.tensor_tensor(cE[:], w1[:], e1[:], AluOp.mult)
        t2 = vtile("t2")
        nc.vector.tensor_tensor(t2[:], w2[:], e2[:], AluOp.mult)
        nc.vector.tensor_tensor(cE[:], cE[:], t2[:], AluOp.add)

        # indices out (int32); token t = 128*j + p
        ii = rsb.tile([128, TOPK, TJ], I32, tag="ii")
        nc.vector.tensor_copy(ii[:, 0, :], idx1[:])
        nc.vector.tensor_copy(ii[:, 1, :], idx2[:])
        nc.sync.dma_start(
            bass.AP(idx_d.tensor, 0, [[TOPK, 128], [1, TOPK], [128 * TOPK, TJ]]),
            ii[:])

        # c -> DRAM (token order) -> partition-broadcast [128, T]
        c_dram = dram.tile([T], F32, tag="cdram")
        cda = c_dram[:]
        nc.sync.dma_start(
            bass.AP(cda.tensor, cda.offset, [[1, 128], [128, TJ]]), cE[:])
        nc.sync.dma_start(
            cbc[:], bass.AP(cda.tensor, cda.offset, [[0, 128], [1, T]]))
        tc.cur_wait_ts = None

    # ---------------- main dense pipeline ----------------
    rwg_r = rwg_d.rearrange("(c p) i -> p c i", p=128)
    rwu_r = rwu_d.rearrange("(c p) i -> p c i", p=128)
    rwd_r = rwd_d.rearrange("(c p) h -> p c h", p=128)

    # shared-expert tile: U projection + A product (G accumulated above)
    for n in range(4):
        tsl = slice(512 * n, 512 * (n + 1))
        ups = pp.tile([128, 512], F32, tag="ps", name="ups8")
        for k in range(HC):
            nc.tensor.matmul(ups[:], wu8[:, k, :], x32r[:, k, tsl],
                             start=(k == 0), stop=(k == HC - 1))
        silu_t = spool.tile([128, 512], F32, tag="silu", name="silu8")
        nc.scalar.activation(silu_t[:], g8s[n][:], Act.Silu)
        nc.vector.tensor_tensor(a_sb[:, IC, tsl], ups[:], silu_t[:], AluOp.mult)

    # stage A/B for the routed expert: A = silu(G) * U (the combine weight
    # is applied at stage C where it factors out of the I-chunk sum)
    for m in range(IC):
        wg = wpool.tile([128, HC, 128], F32R, tag="w")
        wu = wpool.tile([128, HC, 128], F32R, tag="w")
        with tc.tile_wait_until(0.022 + 0.010 * m):
            nc.sync.dma_start(wg[:], rwg_r[:, :, 128 * m:128 * (m + 1)].bitcast(F32R))
            nc.sync.dma_start(wu[:], rwu_r[:, :, 128 * m:128 * (m + 1)].bitcast(F32R))
        for n in range(4):
            gps = pp.tile([128, 512], F32, tag="ps")
            ups = pp.tile([128, 512], F32, tag="ps")
            tsl = slice(512 * n, 512 * (n + 1))
            for k in range(HC):
                nc.tensor.matmul(gps[:], wg[:, k, :], x32r[:, k, tsl],
                                 start=(k == 0), stop=(k == HC - 1))
            for k in range(HC):
                nc.tensor.matmul(ups[:], wu[:, k, :], x32r[:, k, tsl],
                                 start=(k == 0), stop=(k == HC - 1))
            silu_t = spool.tile([128, 512], F32, tag="silu")
            nc.scalar.activation(silu_t[:], gps[:], Act.Silu)
            nc.vector.tensor_tensor(a_sb[:, m, tsl], ups[:], silu_t[:],
                                    AluOp.mult)

    # stage C: down-projection, Y^T = sum_m Wd[m].T @ A[m]
    ypool = ctx.enter_context(tc.tile_pool(name="y", bufs=3))
    for hh in range(HC):
        wd = wpool.tile([128, IC, 128], F32R, tag="w")
        wds = wshp.tile([128, 128], F32R, tag="wsh")
        with tc.tile_wait_until(0.130 + 0.008 * hh):
            nc.sync.dma_start(wd[:], rwd_r[:, :, 128 * hh:128 * (hh + 1)].bitcast(F32R))
            nc.sync.dma_start(wds[:], swd_d[0:128, 128 * hh:128 * (hh + 1)].bitcast(F32R))
        for n in range(4):
            yr = pp.tile([128, 512], F32, tag="ps")
            ys = pp.tile([128, 512], F32, tag="ps")
            tsl = slice(512 * n, 512 * (n + 1))
            for m in range(IC):
                nc.tensor.matmul(yr[:], wd[:, m, :], a_sb[:, m, tsl],
                                 start=(m == 0), stop=(m == IC - 1))
            nc.tensor.matmul(ys[:], wds[:], a_sb[:, IC, tsl],
                             start=True, stop=True)
            # combine: out = c * routed + shared
            ysb = ypool.tile([128, 512], F32, tag="y")
            nc.vector.tensor_tensor(ysb[:], yr[:], cbc[:, tsl], AluOp.mult)
            nc.vector.tensor_tensor(ysb[:], ysb[:], ys[:], AluOp.add)
            nc.sync.dma_start(outT_d[128 * hh:128 * (hh + 1), tsl], ysb[:])


_NC = None


def _get_nc():
    global _NC
    if _NC is None:
        nc = bacc.Bacc("TRN2", target_bir_lowering=False, debug=False)
        with tile.TileContext(nc) as tc, ExitStack() as ctx:
            _emit(nc, tc, ctx)
        nc.compile()
        _NC = nc
    return _NC


LAST_RESULT = None


def kernel(x, gate_w, expert_bias, sw_gate, sw_up, sw_down,
           rw_gate, rw_up, rw_down, _trace=False):
    global LAST_RESULT
    x = np.asarray(x, np.float32)
    gate_w = np.asarray(gate_w, np.float32)
    expert_bias = np.asarray(expert_bias, np.float32)
    sw_gate = np.asarray(sw_gate, np.float32)
    sw_up = np.asarray(sw_up, np.float32)
    sw_down = np.asarray(sw_down, np.float32)
    rw_gate = np.asarray(rw_gate, np.float32)
    rw_up = np.asarray(rw_up, np.float32)
    rw_down = np.asarray(rw_down, np.float32)

    xT = np.ascontiguousarray(x.reshape(T, H).T)
    gwT = np.ascontiguousarray(gate_w.T)
    tie = np.arange(E, dtype=np.float32) * np.float32(1e-6)
    biastie = (expert_bias + tie).astype(np.float32)
    arangeE = np.arange(E, dtype=np.float32)
    eye = np.eye(E, dtype=np.float32)

    in_maps = []
    for e in range(NCORES):
        in_maps.append({
            "xT": xT,
            "rwg": np.ascontiguousarray(rw_gate[e]),
            "rwu": np.ascontiguousarray(rw_up[e]),
            "rwd": np.ascontiguousarray(rw_down[e]),
            "swg": np.ascontiguousarray(sw_gate[:, 128 * e:128 * (e + 1)]),
            "swu": np.ascontiguousarray(sw_up[:, 128 * e:128 * (e + 1)]),
            "swd": np.ascontiguousarray(sw_down[128 * e:128 * (e + 1), :]),
            "gwT": gwT,
            "biastie": biastie,
            "arangeE": arangeE,
            "onehot": eye[e],
        })

    nc = _get_nc()
    res = run_bass_kernel_spmd(nc, in_maps, core_ids=list(range(NCORES)),
                               trace=_trace)
    LAST_RESULT = res

    acc = np.zeros([H, T], np.float32)
    for e in range(NCORES):
        acc += res.results[e]["outT"]
    out = np.ascontiguousarray(acc.T).reshape(B, S, H)
    idx = res.results[0]["idx"].reshape(B, S, TOPK).astype(np.int32)
    return out, idx


# revision 69
# speedup vs baseline: 1.0535x; 1.0535x over previous
"""DeepSeekMoE Trainium2 kernel (8 NeuronCores, expert-parallel).

Design:
  - Core e owns routed expert e (dense over all T tokens; the combine weight
    c[t,e] zeros out tokens not routed to e, which matches the reference math
    exactly) plus a 128-wide I-slice of the shared expert.
  - All activations are kept transposed (feature-major, [feat, tokens]) so
    every matmul streams tokens as the moving operand with expert weights
    stationary.
  - Router (logits -> sigmoid -> top-2 -> combine weights) runs on-device in
    fp32 (exact); the heavy matmuls run in float32r (~10-bit mantissa,
    4x faster than fp32 on the PE).
  - Host only shards/reassembles: x transpose, per-core weight slices, final
    sum over the 8 partial outputs (each core's partial covers disjoint
    experts / I-slices, so the host sum is exact fp32 adds).
"""
import numpy as np
from contextlib import ExitStack

import concourse.bass as bass
import concourse.bacc as bacc
import concourse.tile as tile
from concourse import masks, mybir
from concourse.bass_utils import run_bass_kernel_spmd

F32 = mybir.dt.float32
F32R = mybir.dt.float32r
I32 = mybir.dt.int32

B, S, H, I, E = 2, 1024, 1024, 1024, 8
T = B * S
HC = H // 128          # contraction chunks for H
IC = I // 128          # I chunks (routed expert M-tiles)
MT = IC + 1            # M-tiles: 8 routed I-chunks + 1 shared-expert slice
TJ = T // 128          # 16 token tiles (for the router transpose)
NCORES = 8
TOPK = 2


def _emit(nc, tc, ctx):
    AluOp = mybir.AluOpType
    Act = mybir.ActivationFunctionType

    xT_d = nc.dram_tensor("xT", [H, T], F32, kind="ExternalInput").ap()
    rwg_d = nc.dram_tensor("rwg", [H, I], F32, kind="ExternalInput").ap()
    rwu_d = nc.dram_tensor("rwu", [H, I], F32, kind="ExternalInput").ap()
    rwd_d = nc.dram_tensor("rwd", [I, H], F32, kind="ExternalInput").ap()
    swg_d = nc.dram_tensor("swg", [H, 128], F32, kind="ExternalInput").ap()
    swu_d = nc.dram_tensor("swu", [H, 128], F32, kind="ExternalInput").ap()
    swd_d = nc.dram_tensor("swd", [128, H], F32, kind="ExternalInput").ap()
    gwT_d = nc.dram_tensor("gwT", [H, E], F32, kind="ExternalInput").ap()
    bt_d = nc.dram_tensor("biastie", [E], F32, kind="ExternalInput").ap()
    ar_d = nc.dram_tensor("arangeE", [E], F32, kind="ExternalInput").ap()
    oh_d = nc.dram_tensor("onehot", [E], F32, kind="ExternalInput").ap()

    outT_d = nc.dram_tensor("outT", [H, T], F32, kind="ExternalOutput").ap()
    idx_d = nc.dram_tensor("idx", [T, TOPK], I32, kind="ExternalOutput").ap()

    # ---------------- persistent pools ----------------
    big = ctx.enter_context(tc.tile_pool(name="big", bufs=1))
    wpool = ctx.enter_context(tc.tile_pool(name="w", bufs=4))
    wshp = ctx.enter_context(tc.tile_pool(name="wsh", bufs=3))
    spool = ctx.enter_context(tc.tile_pool(name="silu", bufs=3))
    dram = ctx.enter_context(tc.tile_pool(name="dram", bufs=1, space="DRAM"))
    # single shared PSUM pool: 8 x 1-bank slots, shared by router and main
    # stages so main matmuls can use banks the router isn't holding
    pp = ctx.enter_context(tc.tile_pool(name="ps", bufs=8, space="PSUM"))

    x32r = big.tile([128, HC, T], F32R, tag="x32r")
    a_sb = big.tile([128, MT, T], F32R, tag="A")
    cbc = big.tile([128, T], F32, tag="cbc")
    ident = big.tile([128, 128], F32, tag="ident")
    masks.make_identity(nc, ident[:])

    # ---------------- router (fp32, exact) ----------------
    # x is DMA'd once as fp32 chunks; the router consumes them directly and
    # the DVE converts each chunk into the resident f32r copy for the main
    # matmul stages. The shared-expert G accumulation rides along inside the
    # chunk loop so the PE has main-pipeline work while x streams.
    wg8 = wpool.tile([128, HC, 128], F32R, tag="w", name="wg8")
    wu8 = wpool.tile([128, HC, 128], F32R, tag="w", name="wu8")
    swg_r0 = swg_d.rearrange("(c p) i -> p c i", p=128)
    swu_r0 = swu_d.rearrange("(c p) i -> p c i", p=128)
    with tc.tile_wait_until(0.012):
        nc.sync.dma_start(wg8[:], swg_r0.bitcast(F32R))
        nc.sync.dma_start(wu8[:], swu_r0.bitcast(F32R))
    g8s = [pp.tile([128, 512], F32, tag="ps", name=f"g8_{n}") for n in range(4)]
    with ExitStack() as rctx:
        rsb = rctx.enter_context(tc.tile_pool(name="rsb", bufs=1))
        xcp = rctx.enter_context(tc.tile_pool(name="xc", bufs=4))

        # Router logits in f32r at fp32 accuracy via hi/lo splitting:
        #   logits = gwh.xh + gwh.xl + gwl.xh   (+gwl.xl ~ 1e-8, dropped)
        # where xh/gwh are the f32r-rounded values and xl/gwl the residuals.
        # Each f32r pass runs at 4x the fp32 matmul rate.
        gw_sb = rsb.tile([128, HC, E], F32, tag="gw")
        gwh = rsb.tile([128, HC, E], F32R, tag="gwh")
        gwl = rsb.tile([128, HC, E], F32R, tag="gwl")
        with tc.high_priority():
            nc.sync.dma_start(gw_sb[:], gwT_d.rearrange("(c p) e -> p c e", p=128))
            nc.sync.dma_start(gwh[:],
                              gwT_d.rearrange("(c p) e -> p c e", p=128).bitcast(F32R))
        nc.vector.tensor_tensor(gwl[:], gw_sb[:], gwh[:], AluOp.subtract)
        bt_sb = rsb.tile([128, E], F32, tag="bt")
        nc.sync.dma_start(bt_sb[:], bass.AP(bt_d.tensor, 0, [[0, 128], [1, E]]))
        ar_sb = rsb.tile([128, E], F32, tag="ar")
        nc.sync.dma_start(ar_sb[:], bass.AP(ar_d.tensor, 0, [[0, 128], [1, E]]))
        oh_sb = rsb.tile([128, E], F32, tag="oh")
        nc.sync.dma_start(oh_sb[:], bass.AP(oh_d.tensor, 0, [[0, 128], [1, E]]))

        # logits (E-major) accumulated over H chunks; x streamed in fp32
        scoE = rsb.tile([8, T], F32, tag="scoE")
        lps = [pp.tile([8, 512], F32, tag="ps", name=f"lp{n}") for n in range(4)]
        for k in range(HC):
            for hf in range(2):
                xc = xcp.tile([128, T // 2], F32, tag="xc")
                csl = slice(1024 * hf, 1024 * (hf + 1))
                with tc.high_priority() if k == 0 else ExitStack():
                    nc.sync.dma_start(xc[:], xT_d[128 * k:128 * (k + 1), csl])
                # f32r conversion for the main stages (rounds on write),
                # plus the rounding residual xl = x - xh for the router
                nc.vector.tensor_copy(x32r[:, k, csl], xc[:])
                xlt = xcp.tile([128, T // 2], F32R, tag="xl", bufs=2)
                nc.vector.tensor_tensor(xlt[:], xc[:], x32r[:, k, csl],
                                        AluOp.subtract)
                for nn in range(2):
                    n = 2 * hf + nn
                    xh = x32r[:, k, 512 * (2 * hf + nn):512 * (2 * hf + nn + 1)]
                    xl = xlt[:, 512 * nn:512 * (nn + 1)]
                    nc.tensor.matmul(lps[n][:], gwh[:, k, :], xh,
                                     start=(k == 0), stop=False)
                    nc.tensor.matmul(lps[n][:], gwh[:, k, :], xl,
                                     start=False, stop=False)
                    nc.tensor.matmul(lps[n][:], gwl[:, k, :], xh,
                                     start=False, stop=(k == HC - 1))
            # shared-expert G accumulation rides the stream: PE work that is
            # ready as soon as this chunk's f32r conversion lands
            for n in range(4):
                nc.tensor.matmul(g8s[n][:], wg8[:, k, :],
                                 x32r[:, k, 512 * n:512 * (n + 1)],
                                 start=(k == 0), stop=(k == HC - 1))
        for n in range(4):
            nc.scalar.activation(scoE[:, 512 * n:512 * (n + 1)], lps[n][:],
                                 Act.Sigmoid)

        # transpose scores to token-major [128, TJ, E]
        sco = rsb.tile([128, TJ, E], F32, tag="sco")
        for j in range(TJ):
            tp = pp.tile([128, 8], F32, tag="ps", name=f"tp{j}")
            nc.tensor.transpose(tp[:], scoE[:, 128 * j:128 * (j + 1)],
                                ident[0:8, 0:8])
            nc.vector.tensor_copy(sco[:, j, :], tp[:])

        def vtile(tag):
            return rsb.tile([128, TJ], F32, tag=tag, name=tag)

        def etile(tag):
            return rsb.tile([128, TJ, E], F32, tag=tag, name=tag)

        def bcE(t):  # [128, TJ] -> [128, TJ, E] free-broadcast
            ap = t[:]
            return bass.AP(ap.tensor, ap.offset, ap.ap + [[0, E]])

        def bcTJ(t):  # [128, E] -> [128, TJ, E] free-broadcast
            ap = t[:]
            return bass.AP(ap.tensor, ap.offset,
                           [ap.ap[0], [0, TJ], ap.ap[1]])

        # Defer the whole top-2 DVE chain: c is only consumed by stage C's
        # combine (~150us in), and scheduling it early starves the DVE work
        # (silu products) that recycles the main pipeline's PSUM slots.
        tc.tile_set_cur_wait(0.085)
        sel = etile("sel")
        nc.vector.tensor_tensor(sel[:], sco[:], bcTJ(bt_sb), AluOp.add)

        max1 = vtile("max1")
        nc.vector.tensor_reduce(max1[:], sel[:], mybir.AxisListType.X, AluOp.max)
        eq1 = etile("eq1")
        nc.vector.tensor_tensor(eq1[:], sel[:], bcE(max1), AluOp.is_ge)
        tmp = etile("tmp")
        nc.vector.tensor_tensor(tmp[:], sco[:], eq1[:], AluOp.mult)
        s1 = vtile("s1")
        nc.vector.tensor_reduce(s1[:], tmp[:], mybir.AxisListType.X, AluOp.max)
        nc.vector.tensor_tensor(tmp[:], bcTJ(ar_sb), eq1[:], AluOp.mult)
        idx1 = vtile("idx1")
        nc.vector.tensor_reduce(idx1[:], tmp[:], mybir.AxisListType.X, AluOp.max)

        sel2 = etile("sel2")
        nc.vector.scalar_tensor_tensor(sel2[:], eq1[:], -1e30, sel[:],
                                       AluOp.mult, AluOp.add)
        max2 = vtile("max2")
        nc.vector.tensor_reduce(max2[:], sel2[:], mybir.AxisListType.X, AluOp.max)
        eq2 = etile("eq2")
        nc.vector.tensor_tensor(eq2[:], sel2[:], bcE(max2), AluOp.is_ge)
        nc.vector.tensor_tensor(tmp[:], sco[:], eq2[:], AluOp.mult)
        s2 = vtile("s2")
        nc.vector.tensor_reduce(s2[:], tmp[:], mybir.AxisListType.X, AluOp.max)
        nc.vector.tensor_tensor(tmp[:], bcTJ(ar_sb), eq2[:], AluOp.mult)
        idx2 = vtile("idx2")
        nc.vector.tensor_reduce(idx2[:], tmp[:], mybir.AxisListType.X, AluOp.max)

        # second half of the chain a little later: two short DVE bursts stall
        # the PSUM-recycling products less than one long one
        tc.tile_set_cur_wait(0.095)
        den = vtile("den")
        nc.vector.tensor_tensor(den[:], s1[:], s2[:], AluOp.add)
        nc.vector.tensor_scalar_max(den[:], den[:], 1e-9)
        dinv = vtile("dinv")
        nc.vector.reciprocal(dinv[:], den[:])
        w1 = vtile("w1")
        nc.vector.tensor_tensor(w1[:], s1[:], dinv[:], AluOp.mult)
        w2 = vtile("w2")
        nc.vector.tensor_tensor(w2[:], s2[:], dinv[:], AluOp.mult)

        e1 = vtile("e1")
        nc.vector.tensor_tensor(tmp[:], eq1[:], bcTJ(oh_sb), AluOp.mult)
        nc.vector.tensor_reduce(e1[:], tmp[:], mybir.AxisListType.X, AluOp.max)
        e2 = vtile("e2")
        nc.vector.tensor_tensor(tmp[:], eq2[:], bcTJ(oh_sb), AluOp.mult)
        nc.vector.tensor_reduce(e2[:], tmp[:], mybir.AxisListType.X, AluOp.max)
        cE = vtile("cE")
        nc.vector.tensor_tensor(cE[:], w1[:], e1[:], AluOp.mult)
        t2 = vtile("t2")
        nc.vector.tensor_tensor(t2[:], w2[:], e2[:], AluOp.mult)
        nc.vector.tensor_tensor(cE[:], cE[:], t2[:], AluOp.add)

        # indices out (int32); token t = 128*j + p
        ii = rsb.tile([128, TOPK, TJ], I32, tag="ii")
        nc.vector.tensor_copy(ii[:, 0, :], idx1[:])
        nc.vector.tensor_copy(ii[:, 1, :], idx2[:])
        nc.sync.dma_start(
            bass.AP(idx_d.tensor, 0, [[TOPK, 128], [1, TOPK], [128 * TOPK, TJ]]),
            ii[:])

        # c -> DRAM (token order) -> partition-broadcast [128, T]
        c_dram = dram.tile([T], F32, tag="cdram")
        cda = c_dram[:]
        nc.sync.dma_start(
            bass.AP(cda.tensor, cda.offset, [[1, 128], [128, TJ]]), cE[:])
        nc.sync.dma_start(
            cbc[:], bass.AP(cda.tensor, cda.offset, [[0, 128], [1, T]]))
        tc.cur_wait_ts = None

    # ---------------- main dense pipeline ----------------
    rwg_r = rwg_d.rearrange("(c p) i -> p c i", p=128)
    rwu_r = rwu_d.rearrange("(c p) i -> p c i", p=128)
    rwd_r = rwd_d.rearrange("(c p) h -> p c h", p=128)

    # shared-expert tile: U projection + A product (G accumulated above)
    for n in range(4):
        tsl = slice(512 * n, 512 * (n + 1))
        ups = pp.tile([128, 512], F32, tag="ps", name="ups8")
        for k in range(HC):
            nc.tensor.matmul(ups[:], wu8[:, k, :], x32r[:, k, tsl],
                             start=(k == 0), stop=(k == HC - 1))
        silu_t = spool.tile([128, 512], F32, tag="silu", name="silu8")
        nc.scalar.activation(silu_t[:], g8s[n][:], Act.Silu)
        nc.vector.tensor_tensor(a_sb[:, IC, tsl], ups[:], silu_t[:], AluOp.mult)

    # stage A/B for the routed expert: A = silu(G) * U (the combine weight
    # is applied at stage C where it factors out of the I-chunk sum)
    for m in range(IC):
        wg = wpool.tile([128, HC, 128], F32R, tag="w")
        wu = wpool.tile([128, HC, 128], F32R, tag="w")
        with tc.tile_wait_until(0.022 + 0.010 * m):
            nc.sync.dma_start(wg[:], rwg_r[:, :, 128 * m:128 * (m + 1)].bitcast(F32R))
            nc.sync.dma_start(wu[:], rwu_r[:, :, 128 * m:128 * (m + 1)].bitcast(F32R))
        for n in range(4):
            gps = pp.tile([128, 512], F32, tag="ps")
            ups = pp.tile([128, 512], F32, tag="ps")
            tsl = slice(512 * n, 512 * (n + 1))
            for k in range(HC):
                nc.tensor.matmul(gps[:], wg[:, k, :], x32r[:, k, tsl],
                                 start=(k == 0), stop=(k == HC - 1))
            for k in range(HC):
                nc.tensor.matmul(ups[:], wu[:, k, :], x32r[:, k, tsl],
                                 start=(k == 0), stop=(k == HC - 1))
            silu_t = spool.tile([128, 512], F32, tag="silu")
            nc.scalar.activation(silu_t[:], gps[:], Act.Silu)
            nc.vector.tensor_tensor(a_sb[:, m, tsl], ups[:], silu_t[:],
                                    AluOp.mult)

    # stage C: down-projection, Y^T = sum_m Wd[m].T @ A[m]
    ypool = ctx.enter_context(tc.tile_pool(name="y", bufs=3))
    for hh in range(HC):
        wd = wpool.tile([128, IC, 128], F32R, tag="w")
        wds = wshp.tile([128, 128], F32R, tag="wsh")
        with tc.tile_wait_until(0.130 + 0.008 * hh):
            nc.sync.dma_start(wd[:], rwd_r[:, :, 128 * hh:128 * (hh + 1)].bitcast(F32R))
            nc.sync.dma_start(wds[:], swd_d[0:128, 128 * hh:128 * (hh + 1)].bitcast(F32R))
        for n in range(4):
            yr = pp.tile([128, 512], F32, tag="ps")
            ys = pp.tile([128, 512], F32, tag="ps")
            tsl = slice(512 * n, 512 * (n + 1))
            for m in range(IC):
                nc.tensor.matmul(yr[:], wd[:, m, :], a_sb[:, m, tsl],
                                 start=(m == 0), stop=(m == IC - 1))
            nc.tensor.matmul(ys[:], wds[:], a_sb[:, IC, tsl],
                             start=True, stop=True)
            # combine: out = c * routed + shared
            ysb = ypool.tile([128, 512], F32, tag="y")
            nc.vector.tensor_tensor(ysb[:], yr[:], cbc[:, tsl], AluOp.mult)
            nc.vector.tensor_tensor(ysb[:], ysb[:], ys[:], AluOp.add)
            nc.sync.dma_start(outT_d[128 * hh:128 * (hh + 1), tsl], ysb[:])


_NC = None


def _get_nc():
    global _NC
    if _NC is None:
        nc = bacc.Bacc("TRN2", target_bir_lowering=False, debug=False)
        with tile.TileContext(nc) as tc, ExitStack() as ctx:
            _emit(nc, tc, ctx)
        nc.compile()
        _NC = nc
    return _NC


LAST_RESULT = None


def kernel(x, gate_w, expert_bias, sw_gate, sw_up, sw_down,
           rw_gate, rw_up, rw_down, _trace=False):
    global LAST_RESULT
    x = np.asarray(x, np.float32)
    gate_w = np.asarray(gate_w, np.float32)
    expert_bias = np.asarray(expert_bias, np.float32)
    sw_gate = np.asarray(sw_gate, np.float32)
    sw_up = np.asarray(sw_up, np.float32)
    sw_down = np.asarray(sw_down, np.float32)
    rw_gate = np.asarray(rw_gate, np.float32)
    rw_up = np.asarray(rw_up, np.float32)
    rw_down = np.asarray(rw_down, np.float32)

    xT = np.ascontiguousarray(x.reshape(T, H).T)
    gwT = np.ascontiguousarray(gate_w.T)
    tie = np.arange(E, dtype=np.float32) * np.float32(1e-6)
    biastie = (expert_bias + tie).astype(np.float32)
    arangeE = np.arange(E, dtype=np.float32)
    eye = np.eye(E, dtype=np.float32)

    in_maps = []
    for e in range(NCORES):
        in_maps.append({
            "xT": xT,
            "rwg": np.ascontiguousarray(rw_gate[e]),
            "rwu": np.ascontiguousarray(rw_up[e]),
            "rwd": np.ascontiguousarray(rw_down[e]),
            "swg": np.ascontiguousarray(sw_gate[:, 128 * e:128 * (e + 1)]),
            "swu": np.ascontiguousarray(sw_up[:, 128 * e:128 * (e + 1)]),
            "swd": np.ascontiguousarray(sw_down[128 * e:128 * (e + 1), :]),
            "gwT": gwT,
            "biastie": biastie,
            "arangeE": arangeE,
            "onehot": eye[e],
        })

    nc = _get_nc()
    res = run_bass_kernel_spmd(nc, in_maps, core_ids=list(range(NCORES)),
                               trace=_trace)
    LAST_RESULT = res

    acc = np.zeros([H, T], np.float32)
    for e in range(NCORES):
        acc += res.results[e]["outT"]
    out = np.ascontiguousarray(acc.T).reshape(B, S, H)
    idx = res.results[0]["idx"].reshape(B, S, TOPK).astype(np.int32)
    return out, idx


# revision 70
# speedup vs baseline: 1.1695x; 1.1100x over previous
"""DeepSeekMoE Trainium2 kernel (8 NeuronCores, expert-parallel).

Design:
  - Core e owns routed expert e (dense over all T tokens; the combine weight
    c[t,e] zeros out tokens not routed to e, which matches the reference math
    exactly) plus a 128-wide I-slice of the shared expert.
  - All activations are kept transposed (feature-major, [feat, tokens]) so
    every matmul streams tokens as the moving operand with expert weights
    stationary.
  - Router (logits -> sigmoid -> top-2 -> combine weights) runs on-device in
    fp32 (exact); the heavy matmuls run in float32r (~10-bit mantissa,
    4x faster than fp32 on the PE).
  - Host only shards/reassembles: x transpose, per-core weight slices, final
    sum over the 8 partial outputs (each core's partial covers disjoint
    experts / I-slices, so the host sum is exact fp32 adds).
"""
import numpy as np
from contextlib import ExitStack

import concourse.bass as bass
import concourse.bacc as bacc
import concourse.tile as tile
from concourse import masks, mybir
from concourse.bass_utils import run_bass_kernel_spmd

F32 = mybir.dt.float32
F32R = mybir.dt.float32r
I32 = mybir.dt.int32

B, S, H, I, E = 2, 1024, 1024, 1024, 8
T = B * S
HC = H // 128          # contraction chunks for H
IC = I // 128          # I chunks (routed expert M-tiles)
MT = IC + 1            # M-tiles: 8 routed I-chunks + 1 shared-expert slice
TJ = T // 128          # 16 token tiles (for the router transpose)
NCORES = 8
TOPK = 2


def _emit(nc, tc, ctx):
    AluOp = mybir.AluOpType
    Act = mybir.ActivationFunctionType

    xT_d = nc.dram_tensor("xT", [H, T], F32, kind="ExternalInput").ap()
    rwg_d = nc.dram_tensor("rwg", [H, I], F32, kind="ExternalInput").ap()
    rwu_d = nc.dram_tensor("rwu", [H, I], F32, kind="ExternalInput").ap()
    rwd_d = nc.dram_tensor("rwd", [I, H], F32, kind="ExternalInput").ap()
    swg_d = nc.dram_tensor("swg", [H, 128], F32, kind="ExternalInput").ap()
    swu_d = nc.dram_tensor("swu", [H, 128], F32, kind="ExternalInput").ap()
    swd_d = nc.dram_tensor("swd", [128, H], F32, kind="ExternalInput").ap()
    gwT_d = nc.dram_tensor("gwT", [H, E], F32, kind="ExternalInput").ap()
    bt_d = nc.dram_tensor("biastie", [E], F32, kind="ExternalInput").ap()
    ar_d = nc.dram_tensor("arangeE", [E], F32, kind="ExternalInput").ap()
    oh_d = nc.dram_tensor("onehot", [E], F32, kind="ExternalInput").ap()

    outT_d = nc.dram_tensor("outT", [H, T], F32, kind="ExternalOutput").ap()
    idx_d = nc.dram_tensor("idx", [T, TOPK], I32, kind="ExternalOutput").ap()

    # ---------------- persistent pools ----------------
    big = ctx.enter_context(tc.tile_pool(name="big", bufs=1))
    wpool = ctx.enter_context(tc.tile_pool(name="w", bufs=4))
    wshp = ctx.enter_context(tc.tile_pool(name="wsh", bufs=3))
    spool = ctx.enter_context(tc.tile_pool(name="silu", bufs=3))
    dram = ctx.enter_context(tc.tile_pool(name="dram", bufs=1, space="DRAM"))
    # single shared PSUM pool: 8 x 1-bank slots, shared by router and main
    # stages so main matmuls can use banks the router isn't holding
    pp = ctx.enter_context(tc.tile_pool(name="ps", bufs=8, space="PSUM"))

    x32r = big.tile([128, HC, T], F32R, tag="x32r")
    a_sb = big.tile([128, MT, T], F32R, tag="A")
    cbc = big.tile([128, T], F32, tag="cbc")
    ident = big.tile([128, 128], F32, tag="ident")
    masks.make_identity(nc, ident[:])

    # ---------------- router (fp32, exact) ----------------
    # x is DMA'd once as fp32 chunks; the router consumes them directly and
    # the DVE converts each chunk into the resident f32r copy for the main
    # matmul stages. The shared-expert G accumulation rides along inside the
    # chunk loop so the PE has main-pipeline work while x streams.
    wg8 = wpool.tile([128, HC, 128], F32R, tag="w", name="wg8")
    wu8 = wpool.tile([128, HC, 128], F32R, tag="w", name="wu8")
    swg_r0 = swg_d.rearrange("(c p) i -> p c i", p=128)
    swu_r0 = swu_d.rearrange("(c p) i -> p c i", p=128)
    with tc.tile_wait_until(0.012):
        nc.sync.dma_start(wg8[:], swg_r0.bitcast(F32R))
        nc.sync.dma_start(wu8[:], swu_r0.bitcast(F32R))
    g8s = [pp.tile([128, 512], F32, tag="ps", name=f"g8_{n}") for n in range(4)]
    with ExitStack() as rctx:
        rsb = rctx.enter_context(tc.tile_pool(name="rsb", bufs=1))
        xcp = rctx.enter_context(tc.tile_pool(name="xc", bufs=4))

        # Router logits in f32r at fp32 accuracy via hi/lo splitting:
        #   logits = gwh.xh + gwh.xl + gwl.xh   (+gwl.xl ~ 1e-8, dropped)
        # where xh/gwh are the f32r-rounded values and xl/gwl the residuals.
        # Each f32r pass runs at 4x the fp32 matmul rate.
        gw_sb = rsb.tile([128, HC, E], F32, tag="gw")
        gwh = rsb.tile([128, HC, E], F32R, tag="gwh")
        gwl = rsb.tile([128, HC, E], F32R, tag="gwl")
        with tc.high_priority():
            nc.sync.dma_start(gw_sb[:], gwT_d.rearrange("(c p) e -> p c e", p=128))
            nc.sync.dma_start(gwh[:],
                              gwT_d.rearrange("(c p) e -> p c e", p=128).bitcast(F32R))
        nc.vector.tensor_tensor(gwl[:], gw_sb[:], gwh[:], AluOp.subtract)
        bt_sb = rsb.tile([128, E], F32, tag="bt")
        nc.sync.dma_start(bt_sb[:], bass.AP(bt_d.tensor, 0, [[0, 128], [1, E]]))
        ar_sb = rsb.tile([128, E], F32, tag="ar")
        nc.sync.dma_start(ar_sb[:], bass.AP(ar_d.tensor, 0, [[0, 128], [1, E]]))
        oh_sb = rsb.tile([128, E], F32, tag="oh")
        nc.sync.dma_start(oh_sb[:], bass.AP(oh_d.tensor, 0, [[0, 128], [1, E]]))

        # logits (E-major) accumulated over H chunks; x streamed in fp32
        scoE = rsb.tile([8, T], F32, tag="scoE")
        lps = [pp.tile([8, 512], F32, tag="ps", name=f"lp{n}") for n in range(4)]
        for k in range(HC):
            for hf in range(2):
                xc = xcp.tile([128, T // 2], F32, tag="xc")
                csl = slice(1024 * hf, 1024 * (hf + 1))
                with tc.high_priority() if k == 0 else ExitStack():
                    nc.sync.dma_start(xc[:], xT_d[128 * k:128 * (k + 1), csl])
                # f32r conversion for the main stages (rounds on write),
                # plus the rounding residual xl = x - xh for the router
                nc.vector.tensor_copy(x32r[:, k, csl], xc[:])
                xlt = xcp.tile([128, T // 2], F32R, tag="xl", bufs=2)
                nc.vector.tensor_tensor(xlt[:], xc[:], x32r[:, k, csl],
                                        AluOp.subtract)
                for nn in range(2):
                    n = 2 * hf + nn
                    xh = x32r[:, k, 512 * (2 * hf + nn):512 * (2 * hf + nn + 1)]
                    xl = xlt[:, 512 * nn:512 * (nn + 1)]
                    nc.tensor.matmul(lps[n][:], gwh[:, k, :], xh,
                                     start=(k == 0), stop=False)
                    nc.tensor.matmul(lps[n][:], gwh[:, k, :], xl,
                                     start=False, stop=False)
                    nc.tensor.matmul(lps[n][:], gwl[:, k, :], xh,
                                     start=False, stop=(k == HC - 1))
            # shared-expert G accumulation rides the stream: PE work that is
            # ready as soon as this chunk's f32r conversion lands
            for n in range(4):
                nc.tensor.matmul(g8s[n][:], wg8[:, k, :],
                                 x32r[:, k, 512 * n:512 * (n + 1)],
                                 start=(k == 0), stop=(k == HC - 1))
        for n in range(4):
            nc.scalar.activation(scoE[:, 512 * n:512 * (n + 1)], lps[n][:],
                                 Act.Sigmoid)

        # transpose scores to token-major [128, TJ, E]
        sco = rsb.tile([128, TJ, E], F32, tag="sco")
        for j in range(TJ):
            tp = pp.tile([128, 8], F32, tag="ps", name=f"tp{j}")
            nc.tensor.transpose(tp[:], scoE[:, 128 * j:128 * (j + 1)],
                                ident[0:8, 0:8])
            nc.vector.tensor_copy(sco[:, j, :], tp[:])

        def vtile(tag):
            return rsb.tile([128, TJ], F32, tag=tag, name=tag)

        def etile(tag):
            return rsb.tile([128, TJ, E], F32, tag=tag, name=tag)

        def bcE(t):  # [128, TJ] -> [128, TJ, E] free-broadcast
            ap = t[:]
            return bass.AP(ap.tensor, ap.offset, ap.ap + [[0, E]])

        def bcTJ(t):  # [128, E] -> [128, TJ, E] free-broadcast
            ap = t[:]
            return bass.AP(ap.tensor, ap.offset,
                           [ap.ap[0], [0, TJ], ap.ap[1]])

        # Defer the whole top-2 DVE chain: c is only consumed by stage C's
        # combine (~150us in), and scheduling it early starves the DVE work
        # (silu products) that recycles the main pipeline's PSUM slots.
        tc.tile_set_cur_wait(0.085)
        sel = etile("sel")
        nc.vector.tensor_tensor(sel[:], sco[:], bcTJ(bt_sb), AluOp.add)

        max1 = vtile("max1")
        nc.vector.tensor_reduce(max1[:], sel[:], mybir.AxisListType.X, AluOp.max)
        eq1 = etile("eq1")
        nc.vector.tensor_tensor(eq1[:], sel[:], bcE(max1), AluOp.is_ge)
        tmp = etile("tmp")
        nc.vector.tensor_tensor(tmp[:], sco[:], eq1[:], AluOp.mult)
        s1 = vtile("s1")
        nc.vector.tensor_reduce(s1[:], tmp[:], mybir.AxisListType.X, AluOp.max)
        nc.vector.tensor_tensor(tmp[:], bcTJ(ar_sb), eq1[:], AluOp.mult)
        idx1 = vtile("idx1")
        nc.vector.tensor_reduce(idx1[:], tmp[:], mybir.AxisListType.X, AluOp.max)

        sel2 = etile("sel2")
        nc.vector.scalar_tensor_tensor(sel2[:], eq1[:], -1e30, sel[:],
                                       AluOp.mult, AluOp.add)
        max2 = vtile("max2")
        nc.vector.tensor_reduce(max2[:], sel2[:], mybir.AxisListType.X, AluOp.max)
        eq2 = etile("eq2")
        nc.vector.tensor_tensor(eq2[:], sel2[:], bcE(max2), AluOp.is_ge)
        nc.vector.tensor_tensor(tmp[:], sco[:], eq2[:], AluOp.mult)
        s2 = vtile("s2")
        nc.vector.tensor_reduce(s2[:], tmp[:], mybir.AxisListType.X, AluOp.max)
        nc.vector.tensor_tensor(tmp[:], bcTJ(ar_sb), eq2[:], AluOp.mult)
        idx2 = vtile("idx2")
        nc.vector.tensor_reduce(idx2[:], tmp[:], mybir.AxisListType.X, AluOp.max)

        den = vtile("den")
        nc.vector.tensor_tensor(den[:], s1[:], s2[:], AluOp.add)
        nc.vector.tensor_scalar_max(den[:], den[:], 1e-9)
        dinv = vtile("dinv")
        nc.vector.reciprocal(dinv[:], den[:])
        w1 = vtile("w1")
        nc.vector.tensor_tensor(w1[:], s1[:], dinv[:], AluOp.mult)
        w2 = vtile("w2")
        nc.vector.tensor_tensor(w2[:], s2[:], dinv[:], AluOp.mult)

        e1 = vtile("e1")
        nc.vector.tensor_tensor(tmp[:], eq1[:], bcTJ(oh_sb), AluOp.mult)
        nc.vector.tensor_reduce(e1[:], tmp[:], mybir.AxisListType.X, AluOp.max)
        e2 = vtile("e2")
        nc.vector.tensor_tensor(tmp[:], eq2[:], bcTJ(oh_sb), AluOp.mult)
        nc.vector.tensor_reduce(e2[:], tmp[:], mybir.AxisListType.X, AluOp.max)
        cE = vtile("cE")
        nc.vector.tensor_tensor(cE[:], w1[:], e1[:], AluOp.mult)
        t2 = vtile("t2")
        nc.vector.tensor_tensor(t2[:], w2[:], e2[:], AluOp.mult)
        nc.vector.tensor_tensor(cE[:], cE[:], t2[:], AluOp.add)

        # indices out (int32); token t = 128*j + p
        ii = rsb.tile([128, TOPK, TJ], I32, tag="ii")
        nc.vector.tensor_copy(ii[:, 0, :], idx1[:])
        nc.vector.tensor_copy(ii[:, 1, :], idx2[:])
        nc.sync.dma_start(
            bass.AP(idx_d.tensor, 0, [[TOPK, 128], [1, TOPK], [128 * TOPK, TJ]]),
            ii[:])

        # c -> DRAM (token order) -> partition-broadcast [128, T]
        c_dram = dram.tile([T], F32, tag="cdram")
        cda = c_dram[:]
        nc.sync.dma_start(
            bass.AP(cda.tensor, cda.offset, [[1, 128], [128, TJ]]), cE[:])
        nc.sync.dma_start(
            cbc[:], bass.AP(cda.tensor, cda.offset, [[0, 128], [1, T]]))
        tc.cur_wait_ts = None

    # ---------------- main dense pipeline ----------------
    rwg_r = rwg_d.rearrange("(c p) i -> p c i", p=128)
    rwu_r = rwu_d.rearrange("(c p) i -> p c i", p=128)
    rwd_r = rwd_d.rearrange("(c p) h -> p c h", p=128)

    # shared-expert tile: U projection + A product (G accumulated above)
    for n in range(4):
        tsl = slice(512 * n, 512 * (n + 1))
        ups = pp.tile([128, 512], F32, tag="ps", name="ups8")
        for k in range(HC):
            nc.tensor.matmul(ups[:], wu8[:, k, :], x32r[:, k, tsl],
                             start=(k == 0), stop=(k == HC - 1))
        silu_t = spool.tile([128, 512], F32, tag="silu", name="silu8")
        nc.scalar.activation(silu_t[:], g8s[n][:], Act.Silu)
        nc.vector.tensor_tensor(a_sb[:, IC, tsl], ups[:], silu_t[:], AluOp.mult)

    # stage A/B for the routed expert: A = silu(G) * U (the combine weight
    # is applied at stage C where it factors out of the I-chunk sum)
    for m in range(IC):
        wg = wpool.tile([128, HC, 128], F32R, tag="w")
        wu = wpool.tile([128, HC, 128], F32R, tag="w")
        with tc.tile_wait_until(0.022 + 0.010 * m):
            nc.sync.dma_start(wg[:], rwg_r[:, :, 128 * m:128 * (m + 1)].bitcast(F32R))
            nc.sync.dma_start(wu[:], rwu_r[:, :, 128 * m:128 * (m + 1)].bitcast(F32R))
        for n in range(4):
            gps = pp.tile([128, 512], F32, tag="ps")
            ups = pp.tile([128, 512], F32, tag="ps")
            tsl = slice(512 * n, 512 * (n + 1))
            for k in range(HC):
                nc.tensor.matmul(gps[:], wg[:, k, :], x32r[:, k, tsl],
                                 start=(k == 0), stop=(k == HC - 1))
            for k in range(HC):
                nc.tensor.matmul(ups[:], wu[:, k, :], x32r[:, k, tsl],
                                 start=(k == 0), stop=(k == HC - 1))
            silu_t = spool.tile([128, 512], F32, tag="silu")
            nc.scalar.activation(silu_t[:], gps[:], Act.Silu)
            nc.vector.tensor_tensor(a_sb[:, m, tsl], ups[:], silu_t[:],
                                    AluOp.mult)

    # stage C: down-projection, Y^T = sum_m Wd[m].T @ A[m]
    ypool = ctx.enter_context(tc.tile_pool(name="y", bufs=3))
    for hh in range(HC):
        wd = wpool.tile([128, IC, 128], F32R, tag="w")
        wds = wshp.tile([128, 128], F32R, tag="wsh")
        with tc.tile_wait_until(0.130 + 0.008 * hh):
            nc.sync.dma_start(wd[:], rwd_r[:, :, 128 * hh:128 * (hh + 1)].bitcast(F32R))
            nc.sync.dma_start(wds[:], swd_d[0:128, 128 * hh:128 * (hh + 1)].bitcast(F32R))
        for n in range(4):
            yr = pp.tile([128, 512], F32, tag="ps")
            ys = pp.tile([128, 512], F32, tag="ps")
            tsl = slice(512 * n, 512 * (n + 1))
            for m in range(IC):
                nc.tensor.matmul(yr[:], wd[:, m, :], a_sb[:, m, tsl],
                                 start=(m == 0), stop=(m == IC - 1))
            nc.tensor.matmul(ys[:], wds[:], a_sb[:, IC, tsl],
                             start=True, stop=True)
            # combine: out = c * routed + shared
            ysb = ypool.tile([128, 512], F32, tag="y")
            nc.vector.tensor_tensor(ysb[:], yr[:], cbc[:, tsl], AluOp.mult)
            nc.vector.tensor_tensor(ysb[:], ysb[:], ys[:], AluOp.add)
            nc.sync.dma_start(outT_d[128 * hh:128 * (hh + 1), tsl], ysb[:])


_NC = None


def _get_nc():
    global _NC
    if _NC is None:
        nc = bacc.Bacc("TRN2", target_bir_lowering=False, debug=False)
        with tile.TileContext(nc) as tc, ExitStack() as ctx:
            _emit(nc, tc, ctx)
        nc.compile()
        _NC = nc
    return _NC


LAST_RESULT = None


def kernel(x, gate_w, expert_bias, sw_gate, sw_up, sw_down,
           rw_gate, rw_up, rw_down, _trace=False):
    global LAST_RESULT
    x = np.asarray(x, np.float32)
    gate_w = np.asarray(gate_w, np.float32)
    expert_bias = np.asarray(expert_bias, np.float32)
    sw_gate = np.asarray(sw_gate, np.float32)
    sw_up = np.asarray(sw_up, np.float32)
    sw_down = np.asarray(sw_down, np.float32)
    rw_gate = np.asarray(rw_gate, np.float32)
    rw_up = np.asarray(rw_up, np.float32)
    rw_down = np.asarray(rw_down, np.float32)

    xT = np.ascontiguousarray(x.reshape(T, H).T)
    gwT = np.ascontiguousarray(gate_w.T)
    tie = np.arange(E, dtype=np.float32) * np.float32(1e-6)
    biastie = (expert_bias + tie).astype(np.float32)
    arangeE = np.arange(E, dtype=np.float32)
    eye = np.eye(E, dtype=np.float32)

    in_maps = []
    for e in range(NCORES):
        in_maps.append({
            "xT": xT,
            "rwg": np.ascontiguousarray(rw_gate[e]),
            "rwu": np.ascontiguousarray(rw_up[e]),
            "rwd": np.ascontiguousarray(rw_down[e]),
            "swg": np.ascontiguousarray(sw_gate[:, 128 * e:128 * (e + 1)]),
            "swu": np.ascontiguousarray(sw_up[:, 128 * e:128 * (e + 1)]),
            "swd": np.ascontiguousarray(sw_down[128 * e:128 * (e + 1), :]),
            "gwT": gwT,
            "biastie": biastie,
            "arangeE": arangeE,
            "onehot": eye[e],
        })

    nc = _get_nc()
    res = run_bass_kernel_spmd(nc, in_maps, core_ids=list(range(NCORES)),
                               trace=_trace)
    LAST_RESULT = res

    acc = np.zeros([H, T], np.float32)
    for e in range(NCORES):
        acc += res.results[e]["outT"]
    out = np.ascontiguousarray(acc.T).reshape(B, S, H)
    idx = res.results[0]["idx"].reshape(B, S, TOPK).astype(np.int32)
    return out, idx
